# revision 45
# baseline (speedup 1.0000x reference)
"""Trainium2 Bass kernel: causal multi-head attention (B=4,S=2048,D=1024,H=16).

Sharding (8 cores, no collectives): core c -> batch b=c//2, q-half h=c%2.
Each core computes all 16 heads for 8 interleaved query tiles of 128 rows
(abs q-tile t = 2*j + h for local slot j), plus full K/V for its batch,
and the full fc_out for its own query rows.  The host scatters the 8
per-core [1024,1024] outputs back into [4,2048,1024].

Device pipeline per core (all matmuls bf16, f32 accumulation), organized
to keep the tensor engine continuously busy (TRN2 PE DVFS reaches 2.4GHz
only after ~3us of uninterrupted execution):

  A: x^T / xq^T via PE transposes (casts/evacuations alternate
     ScalarE/VectorE, DMAs spread over the sync+scalar queues), all
     weight tensors loaded via casting gpsimd DMAs (f32->bf16 in the
     DMA, no staging buffers or cast ops), then a dense V-projection
     pass over all 16 s-tiles with pair-0 K^T/Q^T blocks interleaved.
  C: per head, per k-tile: scores^T into a 2-bank PSUM tile (double
     buffered), ONE exp ACTIVATE per k-tile (narrow adjacent k-tiles
     are paired into a single strided ACTIVATE), 0/1 mask multiply on
     mixed tiles only, out^T accumulation per 512-col group with
     ones-augmented V (row 64 = softmax denominator).  Softmax
     normalization is split: the PSUM-side prep (denominator copy,
     reciprocal, output copy) is emitted as soon as a group's
     accumulation completes (freeing its PSUM bank), while the
     tensor-engine finalize (K=1-matmul reciprocal broadcast +
     multiply) for the last group is deferred into the next head's
     stream so the tensor engine never waits on it.  K^T/Q^T projection
     chunks for pair g+1 (and the Wo casting DMAs) are interleaved into
     pair g's attention stream to fill tensor-engine gaps, and the Exp
     activation table is preloaded at startup.
  D: fc_out = concat^T.T @ Wo + bo for the local query rows.

The program is specialized at build time to the mask's block structure
(skip all-zero blocks / skip masking on all-ones blocks); this is computed
from the actual mask input, so it stays correct for any mask.
"""

import os
import numpy as np
import ml_dtypes

import concourse.bass as bass
import concourse.mybir as mybir
import concourse.tile as tile
from concourse import bacc
from concourse.bass_utils import run_bass_kernel_spmd

B, S, D, H, HD = 4, 2048, 1024, 16, 64
N_CORES = 8
ST = 128               # tile edge (partition size)
NKT = S // ST          # 16 key tiles
NJ = 8                 # local query slots per core (8*128 = 1024 rows)
NDC = D // ST          # 8 contraction chunks
NG = H // 2            # 8 head pairs (2 heads packed per 128 partitions)
NSG = NKT // 4         # 4 s-groups of 512 rows
NQG = NJ // 4          # 2 q s-groups
NB = NJ // 4           # 2 x 512-col output groups of slots

F32 = mybir.dt.float32
BF16 = mybir.dt.bfloat16


def _classify(mask: np.ndarray):
    """Block structure of the mask, unioned over the two q-halves.

    Returns (cls[NJ][NKT] in {0 skip,1 full,2 mixed}, mixed list,
    mixed->dedup-index map, number of distinct mask tiles).
    """
    cls = np.zeros((NJ, NKT), dtype=int)
    for j in range(NJ):
        for k in range(NKT):
            blocks = [
                mask[(2 * j + h) * ST:(2 * j + h + 1) * ST, k * ST:(k + 1) * ST]
                for h in (0, 1)
            ]
            if all((b != 0).all() for b in blocks):
                cls[j, k] = 1
            elif all((b == 0).all() for b in blocks):
                cls[j, k] = 0
            else:
                cls[j, k] = 2
        # close interior holes so every slot's computed k-range is contiguous
        nz = np.nonzero(cls[j])[0]
        if len(nz):
            for k in range(nz[0], nz[-1] + 1):
                if cls[j, k] == 0:
                    cls[j, k] = 2
    mixed = [(j, k) for j in range(NJ) for k in range(NKT) if cls[j, k] == 2]
    # dedup mixed tiles by their (h=0, h=1) block content pair: the causal
    # mask yields only 2 distinct patterns, saving SBUF and DMA
    dedup = {}
    midx = {}
    for (j, k) in mixed:
        key = tuple(
            mask[(2 * j + h) * ST:(2 * j + h + 1) * ST,
                 k * ST:(k + 1) * ST].tobytes() for h in (0, 1))
        if key not in dedup:
            dedup[key] = len(dedup)
        midx[(j, k)] = dedup[key]
    return cls, mixed, midx, max(len(dedup), 1)


def _build(cls, mixed, mixed_idx, n_maskt):
    """Build the (uniform, SPMD) Bass program for one core's shard."""
    nc = bacc.Bacc("TRN2", target_bir_lowering=False, debug=False,
                   num_devices=N_CORES)

    x_d = nc.dram_tensor("x", [S, D], BF16, kind="ExternalInput")
    xq_d = nc.dram_tensor("xq", [NJ * ST, D], BF16, kind="ExternalInput")
    wq_d = nc.dram_tensor("wq", [H, D, HD], F32, kind="ExternalInput")
    wk_d = nc.dram_tensor("wk", [H, D, HD], F32, kind="ExternalInput")
    wv_d = nc.dram_tensor("wv", [H, D, HD], F32, kind="ExternalInput")
    wo_d = nc.dram_tensor("wo", [D, D], F32, kind="ExternalInput")
    bq_d = nc.dram_tensor("bq", [H, HD], F32, kind="ExternalInput")
    bk_d = nc.dram_tensor("bk", [H, HD], F32, kind="ExternalInput")
    bv_d = nc.dram_tensor("bv", [H, HD], F32, kind="ExternalInput")
    bo_d = nc.dram_tensor("bo", [D], F32, kind="ExternalInput")
    mt_d = nc.dram_tensor("maskt", [n_maskt, ST, ST], BF16, kind="ExternalInput")
    out_d = nc.dram_tensor("out", [NJ * ST, D], F32, kind="ExternalOutput")

    slots_k = [[j for j in range(NJ) if cls[j, k]] for k in range(NKT)]
    kfirst = {}
    klast = {}
    for j in range(NJ):
        ks = [k for k in range(NKT) if cls[j, k]]
        if ks:
            kfirst[j], klast[j] = ks[0], ks[-1]

    bank_slots = [[j for j in range(4 * b_, 4 * b_ + 4) if j in kfirst]
                  for b_ in range(NB)]
    bklast = {b_: max(klast[j] for j in bank_slots[b_])
              for b_ in range(NB) if bank_slots[b_]}
    bank_fast = {b_: len({kfirst[j] for j in bank_slots[b_]}) == 1
                 for b_ in range(NB) if bank_slots[b_]}

    from concourse.masks import make_identity

    with tile.TileContext(nc) as tc:
        with (
            tc.tile_pool(name="persist", bufs=1) as pp,
        ):
            # ---- persistent SBUF tensors -------------------------------
            kt_t = [pp.tile([ST, S], BF16, name=f"ktg{g}", tag=f"ktg{g}")
                    for g in range(NG)]
            qt_t = [pp.tile([ST, NJ * ST], BF16, name=f"qtg{g}", tag=f"qtg{g}")
                    for g in range(NG)]
            vb = pp.tile([ST, NKT, H, HD + 1], BF16, name="vb", tag="vb")
            cat = [pp.tile([ST, NJ * ST], BF16, name=f"catg{g}", tag=f"catg{g}")
                   for g in range(NG)]
            ident = pp.tile([ST, ST], BF16, name="ident", tag="ident")
            ones1 = pp.tile([1, HD], BF16, name="ones1", tag="ones1")
            mtb = pp.tile([ST, max(n_maskt, 1), ST], BF16, name="mtb",
                          tag="mtb")

            nc.vector.memset(vb[:, :, :, HD:HD + 1], 1.0)
            nc.vector.memset(ones1[:, :], 1.0)
            junk = pp.tile([1, 1], F32, name="junk", tag="junk")
            nc.vector.memset(junk[:, :], 0.0)
            make_identity(nc, ident[:, :])
            nc.scalar.dma_start(mtb[:, :, :],
                                mt_d.ap().rearrange("m p f -> p m f"))

            def load_bias_pair(pool, bias_d, name):
                # [128, NG] f32: partition = (h%2)*64+e, column = pair idx
                t = pool.tile([ST, NG], F32, name=name, tag=name, bufs=1)
                src = bias_d.ap()
                nc.scalar.dma_start(
                    t[:, :],
                    bass.AP(tensor=src.tensor, offset=src.offset,
                            ap=[[1, ST], [ST, NG]]))
                return t

            # x^T tiles (live until the last K/Q projection)
            xtp_cm = tc.tile_pool(name="xtp", bufs=1, side="right")
            xtp = xtp_cm.__enter__()
            xt_all = xtp.tile([ST, NDC, NKT, ST], BF16, name="xt_all",
                              tag="xt_all")
            xqt_all = xtp.tile([ST, NDC, NJ, ST], BF16, name="xqt_all",
                               tag="xqt_all")

            # weight-pair destination pool (lives phase A .. attention)
            wp_cm = tc.tile_pool(name="wpair", bufs=2)
            wp = wp_cm.__enter__()

            bkp = load_bias_pair(pp, bk_d, "bkp")
            bqp = load_bias_pair(pp, bq_d, "bqp")

            def k_proj_block(wpr, g, sg, pool, pbufs=2):
                psk = pool.tile([ST, 512], F32, tag="psk", name="psk",
                                bufs=pbufs)
                for c in range(NDC):
                    nc.tensor.matmul(
                        psk[:, :], wpr[:, c, :],
                        xt_all[:, c, 4 * sg:4 * (sg + 1), :],
                        start=(c == 0), stop=(c == NDC - 1))
                nc.vector.tensor_scalar(
                    kt_t[g][:, sg * 512:(sg + 1) * 512],
                    psk[:, :], bkp[:, g:g + 1], None,
                    mybir.AluOpType.add)

            def q_proj_block(wpr, g, sg, pool, pbufs=2):
                psk = pool.tile([ST, 512], F32, tag="psk", name="psk",
                                bufs=pbufs)
                for c in range(NDC):
                    nc.tensor.matmul(
                        psk[:, :], wpr[:, c, :],
                        xqt_all[:, c, 4 * sg:4 * (sg + 1), :],
                        start=(c == 0), stop=(c == NDC - 1))
                nc.vector.tensor_scalar(
                    qt_t[g][:, sg * 512:(sg + 1) * 512],
                    psk[:, :], bqp[:, g:g + 1], None,
                    mybir.AluOpType.add)

            def stage_pair_weights(w_d, g, tag):
                # casting gpsimd DMAs straight into the [128, NDC, 128]
                # stationary-pair layout (f32 -> bf16 in the DMA)
                wpr = wp.tile([ST, NDC, ST], BF16, name=f"{tag}{g}", tag=tag)
                for h2 in range(2):
                    src = w_d.ap()[2 * g + h2].rearrange(
                        "(c p) e -> p c e", p=ST)
                    nc.gpsimd.dma_start(
                        wpr[:, :, h2 * HD:(h2 + 1) * HD], src)
                return wpr

            # ---- phase A: x^T, xq^T, then a dense V pass ---------------
            with (
                tc.tile_pool(name="p1a", bufs=2) as p1a,
                tc.tile_pool(name="pv", bufs=1) as pv,
                tc.tile_pool(name="ppst", bufs=3, space="PSUM") as ppst,
                tc.tile_pool(name="ppsv", bufs=2, space="PSUM") as ppsv,
            ):
                wvb = pv.tile([ST, NDC, H, HD], BF16, name="wvb", tag="wvb",
                              bufs=1)
                bvf = pv.tile([ST, H, HD], F32, name="bvf", tag="bvf", bufs=1)
                # pair-0 K/Q weights first on the gpsimd queue (needed
                # ~20us in, before the V weights at ~30us)
                _stage0 = (stage_pair_weights(wk_d, 0, "wkpr"),
                           stage_pair_weights(wq_d, 0, "wqpr"))
                for hh in range(H):
                    srcw = wv_d.ap()[hh].rearrange("(c p) e -> p c e", p=ST)
                    nc.gpsimd.dma_start(wvb[:, :, hh, :], srcw)

                evac_i = 0

                def transpose_tile(dst_all, sti, xsrc_ap):
                    nonlocal evac_i
                    xb = p1a.tile([ST, D], BF16, tag="xb", name="xb",
                                  bufs=6)
                    qeng = nc.sync if (evac_i // 8) % 2 == 0 else nc.scalar
                    qeng.dma_start(xb[:, :], xsrc_ap)
                    for c4 in range(NDC // 4):
                        pst4 = ppst.tile([ST, 4, ST], BF16, tag="pst",
                                         name="pst")
                        for i in range(4):
                            c = 4 * c4 + i
                            nc.tensor.transpose(
                                pst4[:, i, :], xb[:, c * ST:(c + 1) * ST],
                                ident[:, :])
                        if evac_i % 2 == 1:
                            nc.scalar.copy(
                                dst_all[:, 4 * c4:4 * c4 + 4, sti, :],
                                pst4[:, :, :])
                        else:
                            nc.vector.tensor_copy(
                                dst_all[:, 4 * c4:4 * c4 + 4, sti, :],
                                pst4[:, :, :])
                        evac_i += 1

                for st in range(NKT):
                    transpose_tile(xt_all, st,
                                   x_d.ap()[st * ST:(st + 1) * ST, :])
                    if st == 0:
                        srcv = bv_d.ap()
                        nc.scalar.dma_start(
                            bvf[:, :, :],
                            bass.AP(tensor=srcv.tensor, offset=srcv.offset,
                                    ap=[[0, ST]] + list(srcv.ap)))

                for jl in range(NJ):
                    transpose_tile(xqt_all, jl,
                                   xq_d.ap()[jl * ST:(jl + 1) * ST, :])

                # preload the Exp activation table while ScalarE is idle
                nc.scalar.activation(junk[:, :], junk[:, :],
                                     mybir.ActivationFunctionType.Exp)
                _p0 = ([(k_proj_block, _stage0[0], sg, ppst)
                        for sg in range(NSG)] +
                       [(q_proj_block, _stage0[1], sg, ppst)
                        for sg in range(NQG)])

                # dense V projection pass (tensor-heavy, no cross-engine
                # stalls): per s-tile 16 accumulating matmuls + one evac
                for st in range(NKT):
                    psv = ppsv.tile([ST, H * HD], F32, tag="psv", name="psv")
                    for c in range(NDC):
                        for n in range(2):
                            nc.tensor.matmul(
                                psv[:, n * 512:(n + 1) * 512],
                                xt_all[:, c, st, :],
                                wvb[:, c, 8 * n:8 * n + 8, :],
                                start=(c == 0), stop=(c == NDC - 1))
                    nc.vector.tensor_add(
                        vb[:, st, :, 0:HD],
                        psv[:, :].rearrange("p (h e) -> p h e", h=H),
                        bvf[:, :, :])
                    if st % 3 == 2 and _p0:
                        fn, w_, sg_, pl_ = _p0.pop(0)
                        fn(w_, 0, sg_, pl_, 1)
                while _p0:
                    fn, w_, sg_, pl_ = _p0.pop(0)
                    fn(w_, 0, sg_, pl_, 1)

            # ---- phases B/C/D: projections + attention + fc_out --------
            # PSUM budget: psc 2x2 banks + po 2x1 banks + psk 2x1 = 8.
            p2s_cm = tc.tile_pool(name="p2s", bufs=2)
            p2s = p2s_cm.__enter__()
            wop_cm = tc.tile_pool(name="wop", bufs=1)
            wop = wop_cm.__enter__()
            wob = wop.tile([ST, NDC, D], BF16, name="wob", tag="wob")
            bob = wop.tile([ST, D], BF16, name="bob", tag="bob")
            bo_ap = bo_d.ap()
            nc.gpsimd.dma_start(
                bob[:, :],
                bass.AP(tensor=bo_ap.tensor, offset=bo_ap.offset,
                        ap=[[0, ST]] + list(bo_ap.ap)))
            pt_cm = tc.tile_pool(name="ptp", bufs=6)
            ptp = pt_cm.__enter__()
            psc_cm = tc.tile_pool(name="psc", bufs=2, space="PSUM")
            pscp = psc_cm.__enter__()
            po_cm = tc.tile_pool(name="po", bufs=2, space="PSUM")
            pop = po_cm.__enter__()
            psk_cm = tc.tile_pool(name="psk", bufs=2, space="PSUM")
            pskp = psk_cm.__enter__()

            def proj_chunks_for_pair(g, staged=None):
                # closures emitting one tensor-engine chunk each
                if staged is None:
                    wk_pr = stage_pair_weights(wk_d, g, "wkpr")
                    wq_pr = stage_pair_weights(wq_d, g, "wqpr")
                else:
                    wk_pr, wq_pr = staged
                chunks = []
                for sg in range(NSG):
                    chunks.append(
                        lambda sg=sg: k_proj_block(wk_pr, g, sg, pskp))
                for sg in range(NQG):
                    chunks.append(
                        lambda sg=sg: q_proj_block(wq_pr, g, sg, pskp))
                return chunks

            _pair0_chunks = []

            def wo_chunks(cs):
                chunks = []
                for c in cs:
                    def ch(c=c):
                        nc.gpsimd.dma_start(wob[:, c, :],
                                            wo_d.ap()[c * ST:(c + 1) * ST, :])
                    chunks.append(ch)
                return chunks

            # pair 0 was projected during the phase-A V pass

            def attention_head(g, h, pending_chunks, prev_fin):
                """Emit attention for head h (pair g).

                pending_chunks: proj/wo closures drained ~evenly into the
                k-loop.  prev_fin: deferred normalization-finalize closures
                of the previous head, drained after the first k-iterations.
                Returns this head's own finalize closures."""
                r = (h % 2) * HD
                po = {}
                for b_ in range(NB):
                    if bank_slots[b_]:
                        po[b_] = pop.tile([HD + 1, 512], F32, tag="po",
                                          name=f"po{h}_{b_}")
                        if not bank_fast[b_]:
                            nc.vector.memset(po[b_][:, :], 0.0)

                active_ks = [k for k in range(NKT) if slots_k[k]]
                n_it = max(1, (len(active_ks) * 3) // 4)
                drain_every = max(1, n_it // (len(pending_chunks) + 1)) \
                    if pending_chunks else 0

                norm_state = {}

                def norm_pre(b_):
                    # PSUM-side reads: frees the po slot early; no tensor op
                    ltmp = p2s.tile([1, 512], F32, tag="ltmp", name="ltmp",
                                    bufs=1)
                    nc.vector.tensor_copy(ltmp[:, :], po[b_][HD:HD + 1, :])
                    rec = p2s.tile([1, 512], F32, tag="rec", name="rec",
                                   bufs=1)
                    nc.vector.reciprocal_approx_fast(rec[:, :], ltmp[:, :])
                    rec16 = p2s.tile([1, 512], BF16, tag="rec16", name="rec16",
                                     bufs=2)
                    nc.vector.tensor_copy(rec16[:, :], rec[:, :])
                    cslice = cat[g][r:r + HD, 4 * b_ * ST:(4 * b_ + 4) * ST]
                    nc.vector.tensor_copy(cslice, po[b_][0:HD, :])
                    norm_state[b_] = (rec16, cslice)

                def norm_fin(b_):
                    rec16, cslice = norm_state[b_]
                    recps = pskp.tile([HD, 512], F32, tag="psk", name="recps")
                    nc.tensor.matmul(recps[:, :], ones1[:, :], rec16[:, :],
                                     start=True, stop=True)
                    nc.vector.tensor_mul(cslice, cslice, recps[:, :])

                def emit_av(item, paired, pt):
                    done_banks = []
                    for par, (k, runs) in enumerate(item):
                        for run in runs:
                            sub = [run[0]]
                            subs = []
                            for j in run[1:]:
                                if kfirst[j] == kfirst[sub[0]]:
                                    sub.append(j)
                                else:
                                    subs.append(sub)
                                    sub = [j]
                            subs.append(sub)
                            for sub_ in subs:
                                ja, jb = sub_[0], sub_[-1]
                                b_ = ja // 4
                                fast = bank_fast[b_]
                                co = (par * 512 + (ja - 4 * b_) * ST
                                      if paired else ja * ST)
                                nc.tensor.matmul(
                                    po[b_][0:HD + 1,
                                           (ja - 4 * b_) * ST:
                                           (jb + 1 - 4 * b_) * ST],
                                    vb[:, k, h, :],
                                    pt[:, co:co + (jb + 1 - ja) * ST],
                                    start=(fast and k == kfirst[ja]),
                                    stop=(fast and k == bklast[b_]),
                                    skip_group_check=not fast)
                                if fast and k == bklast[b_]:
                                    done_banks.append(b_)
                    return done_banks

                def runs_of(sl):
                    runs = []
                    run = [sl[0]]
                    for j in sl[1:]:
                        if j == run[-1] + 1 and j // 4 == run[0] // 4:
                            run.append(j)
                        else:
                            runs.append(run)
                            run = [j]
                    runs.append(run)
                    return runs

                def is_narrow(runs):
                    return (len(runs) == 1 and
                            (runs[0][-1] - runs[0][0] + 1) * ST <= 512)

                # batch: pair up narrow k-tiles (single run <= 512 wide) so
                # one exp ACTIVATE serves two k-tiles
                items = []
                i = 0
                while i < len(active_ks):
                    k = active_ks[i]
                    rk = runs_of(slots_k[k])
                    if is_narrow(rk) and i + 1 < len(active_ks):
                        k2 = active_ks[i + 1]
                        rk2 = runs_of(slots_k[k2])
                        if is_narrow(rk2):
                            items.append([(k, rk), (k2, rk2)])
                            i += 2
                            continue
                    items.append([(k, rk)])
                    i += 1

                def colof(j, par, paired):
                    # flat column of slot j within psc/pt for this sub-tile
                    if paired:
                        return par * 512 + (j - 4 * (j // 4)) * ST
                    return j * ST

                pending = []
                for ii, item in enumerate(items):
                    paired = len(item) == 2
                    psc = pscp.tile([ST, NJ * ST], F32, tag="psc", name="psc")
                    for par, (k, runs) in enumerate(item):
                        for run in runs:
                            ja, jb = run[0], run[-1]
                            w = (jb + 1 - ja) * ST
                            co = colof(ja, par, paired)
                            nc.tensor.matmul(
                                psc[:, co:co + w],
                                kt_t[g][r:r + HD, k * ST:(k + 1) * ST],
                                qt_t[g][r:r + HD, ja * ST:(jb + 1) * ST],
                                start=True, stop=True)
                    pt = ptp.tile([ST, NJ * ST], BF16, tag="pt", name="pt")
                    if paired:
                        # one exp over both sub-tiles via a strided 3-dim AP
                        o0 = min(colof(k_r[0][0], 0, True)
                                 for (kk, k_r) in item) % 512
                        o1 = max(colof(k_r[0][-1], 0, True) % 512 + ST
                                 for (kk, k_r) in item)
                        psc2 = psc[:, :].rearrange("p (a c) -> p a c", a=2)
                        pt2 = pt[:, :].rearrange("p (a c) -> p a c", a=2)
                        nc.scalar.activation(
                            pt2[:, :, o0:o1], psc2[:, :, o0:o1],
                            mybir.ActivationFunctionType.Exp,
                            scale=1.0 / float(np.sqrt(HD)))
                    else:
                        k, runs = item[0]
                        sl = slots_k[k]
                        jaT, jbT = sl[0], sl[-1]
                        nc.scalar.activation(
                            pt[:, jaT * ST:(jbT + 1) * ST],
                            psc[:, jaT * ST:(jbT + 1) * ST],
                            mybir.ActivationFunctionType.Exp,
                            scale=1.0 / float(np.sqrt(HD)))
                    for par, (k, runs) in enumerate(item):
                        for j in slots_k[k]:
                            if cls[j, k] == 2:
                                m = mixed_idx[(j, k)]
                                co = colof(j, par, paired)
                                nc.vector.tensor_mul(
                                    pt[:, co:co + ST],
                                    pt[:, co:co + ST],
                                    mtb[:, m, :])
                    pending.append((item, paired, pt))
                    if len(pending) > 1:
                        for b_ in emit_av(*pending.pop(0)):
                            norm_pre(b_)
                            if b_ != NB - 1:
                                # groups completing mid-head finalize in-head
                                norm_fin(b_)
                    if prev_fin and ii == 2:
                        while prev_fin:
                            prev_fin.pop(0)()
                    if pending_chunks and drain_every and \
                            ii % drain_every == drain_every - 1:
                        pending_chunks.pop(0)()
                for args in pending:
                    for b_ in emit_av(*args):
                        norm_pre(b_)
                        if b_ != NB - 1:
                            norm_fin(b_)
                while pending_chunks:
                    pending_chunks.pop(0)()
                # slow path for masks where a group never hits bklast (not
                # bank_fast): normalize any group not yet handled
                fins = []
                for b_ in range(NB):
                    if bank_slots[b_] and b_ not in norm_state:
                        norm_pre(b_)
                        if b_ != NB - 1:
                            norm_fin(b_)
                # the last group's tensor finalize is deferred into the
                # next head's stream (returned to the caller)
                if bank_slots[NB - 1]:
                    fins.append(lambda: norm_fin(NB - 1))
                return fins

            fins = []
            for g in range(NG):
                chunks = proj_chunks_for_pair(g + 1) if g + 1 < NG else []
                if g == NG - 3:
                    chunks += wo_chunks(range(0, 4))
                if g == NG - 2:
                    chunks += wo_chunks(range(4, NDC))
                # split interleaved chunks between the two heads
                half = (len(chunks) + 1) // 2
                fins = attention_head(g, 2 * g, chunks[:half], fins)
                fins += attention_head(g, 2 * g + 1, chunks[half:], fins)
            for f in fins:
                f()

            psk_cm.__exit__(None, None, None)
            po_cm.__exit__(None, None, None)
            psc_cm.__exit__(None, None, None)
            pt_cm.__exit__(None, None, None)

            # ---- phase D: fc_out ---------------------------------------
            with (
                tc.tile_pool(name="p3s", bufs=3) as p3s,
                tc.tile_pool(name="psy", bufs=4, space="PSUM") as psy,
            ):
                for jt in range(NJ):
                    py = [psy.tile([ST, 512], F32, tag="py",
                                   name=f"py{jt}_{n}") for n in range(2)]
                    for c in range(NDC):
                        for n in range(2):
                            nc.tensor.matmul(
                                py[n][:, :],
                                cat[c][:, jt * ST:(jt + 1) * ST],
                                wob[:, c, n * 512:(n + 1) * 512],
                                start=(c == 0), stop=(c == NDC - 1))
                    for n in range(2):
                        ysb = p3s.tile([ST, 512], F32, tag="ysb", name="ysb")
                        nc.vector.tensor_add(ysb[:, :], py[n][:, :],
                                             bob[:, n * 512:(n + 1) * 512])
                        nc.sync.dma_start(
                            out_d.ap()[jt * ST:(jt + 1) * ST,
                                       n * 512:(n + 1) * 512],
                            ysb[:, :])

            wop_cm.__exit__(None, None, None)
            p2s_cm.__exit__(None, None, None)
            wp_cm.__exit__(None, None, None)
            xtp_cm.__exit__(None, None, None)

    nc.compile()
    return nc


_CACHE = {}
LAST_RESULT = None


def _get_program(mask):
    key = mask.tobytes()
    if key not in _CACHE:
        cls, mixed, midx, n_maskt = _classify(mask)
        _CACHE[key] = (_build(cls, mixed, midx, n_maskt), cls, mixed, midx,
                       n_maskt)
    return _CACHE[key]


def kernel(x, mask, Wq, bq, Wk, bk, Wv, bv, Wo, bo):
    x = np.ascontiguousarray(np.asarray(x, dtype=np.float32))
    mask = np.asarray(mask)
    nc, cls, mixed, midx, n_maskt = _get_program(mask)

    base = {
        "wq": np.ascontiguousarray(Wq, dtype=np.float32),
        "wk": np.ascontiguousarray(Wk, dtype=np.float32),
        "wv": np.ascontiguousarray(Wv, dtype=np.float32),
        "wo": np.ascontiguousarray(Wo, dtype=np.float32),
        "bq": np.ascontiguousarray(bq, dtype=np.float32),
        "bk": np.ascontiguousarray(bk, dtype=np.float32),
        "bv": np.ascontiguousarray(bv, dtype=np.float32),
        "bo": np.ascontiguousarray(bo, dtype=np.float32),
    }
    in_maps = []
    for c in range(N_CORES):
        b, h = c // 2, c % 2
        qrows = np.concatenate(
            [np.arange((2 * j + h) * ST, (2 * j + h + 1) * ST) for j in range(NJ)])
        mt = np.zeros((n_maskt, ST, ST), dtype=ml_dtypes.bfloat16)
        for (j, k) in mixed:
            blk = mask[(2 * j + h) * ST:(2 * j + h + 1) * ST,
                       k * ST:(k + 1) * ST]
            mt[midx[(j, k)]] = (blk != 0).T.astype(ml_dtypes.bfloat16)
        m = dict(base)
        m["x"] = np.ascontiguousarray(x[b].astype(ml_dtypes.bfloat16))
        m["xq"] = np.ascontiguousarray(
            x[b][qrows].astype(ml_dtypes.bfloat16))
        m["maskt"] = mt
        in_maps.append(m)

    res = run_bass_kernel_spmd(
        nc, in_maps, core_ids=list(range(N_CORES)),
        trace=os.environ.get("BASS_KERNEL_TRACE", "0") == "1")
    global LAST_RESULT
    LAST_RESULT = res

    out = np.empty((B, S, D), dtype=np.float32)
    for c in range(N_CORES):
        b, h = c // 2, c % 2
        oc = res.results[c]["out"]
        for j in range(NJ):
            out[b, (2 * j + h) * ST:(2 * j + h + 1) * ST, :] = \
                oc[j * ST:(j + 1) * ST, :]
    return out


# revision 46
# speedup vs baseline: 1.0113x; 1.0113x over previous
"""Trainium2 Bass kernel: causal multi-head attention (B=4,S=2048,D=1024,H=16).

Sharding (8 cores, no collectives): core c -> batch b=c//2, q-half h=c%2.
Each core computes all 16 heads for 8 interleaved query tiles of 128 rows
(abs q-tile t = 2*j + h for local slot j), plus full K/V for its batch,
and the full fc_out for its own query rows.  The host scatters the 8
per-core [1024,1024] outputs back into [4,2048,1024].

Device pipeline per core (all matmuls bf16, f32 accumulation), organized
to keep the tensor engine continuously busy (TRN2 PE DVFS reaches 2.4GHz
only after ~3us of uninterrupted execution):

  A: x^T / xq^T via PE transposes (casts/evacuations alternate
     ScalarE/VectorE, DMAs spread over the sync+scalar queues), all
     weight tensors loaded via casting gpsimd DMAs (f32->bf16 in the
     DMA, no staging buffers or cast ops), then a dense V-projection
     pass over all 16 s-tiles with pair-0 K^T/Q^T blocks interleaved.
  C: per head, per k-tile: scores^T into a 2-bank PSUM tile (double
     buffered), ONE exp ACTIVATE per k-tile (narrow adjacent k-tiles
     are paired into a single strided ACTIVATE), 0/1 mask multiply on
     mixed tiles only, out^T accumulation per 512-col group with
     ones-augmented V (row 64 = softmax denominator).  Softmax
     normalization is split: the PSUM-side prep (denominator copy,
     reciprocal, output copy) is emitted as soon as a group's
     accumulation completes (freeing its PSUM bank), while the
     tensor-engine finalize (K=1-matmul reciprocal broadcast +
     multiply) for the last group is deferred into the next head's
     stream so the tensor engine never waits on it.  K^T/Q^T projection
     chunks for pair g+1 (and the Wo casting DMAs) are interleaved into
     pair g's attention stream to fill tensor-engine gaps, and the Exp
     activation table is preloaded at startup.
  D: fc_out = concat^T.T @ Wo + bo for the local query rows.

The program is specialized at build time to the mask's block structure
(skip all-zero blocks / skip masking on all-ones blocks); this is computed
from the actual mask input, so it stays correct for any mask.
"""

import os
import numpy as np
import ml_dtypes

import concourse.bass as bass
import concourse.mybir as mybir
import concourse.tile as tile
from concourse import bacc
from concourse.bass_utils import run_bass_kernel_spmd

B, S, D, H, HD = 4, 2048, 1024, 16, 64
N_CORES = 8
ST = 128               # tile edge (partition size)
NKT = S // ST          # 16 key tiles
NJ = 8                 # local query slots per core (8*128 = 1024 rows)
NDC = D // ST          # 8 contraction chunks
NG = H // 2            # 8 head pairs (2 heads packed per 128 partitions)
NSG = NKT // 4         # 4 s-groups of 512 rows
NQG = NJ // 4          # 2 q s-groups
NB = NJ // 4           # 2 x 512-col output groups of slots

F32 = mybir.dt.float32
BF16 = mybir.dt.bfloat16


def _classify(mask: np.ndarray):
    """Block structure of the mask, unioned over the two q-halves.

    Returns (cls[NJ][NKT] in {0 skip,1 full,2 mixed}, mixed list,
    mixed->dedup-index map, number of distinct mask tiles).
    """
    cls = np.zeros((NJ, NKT), dtype=int)
    for j in range(NJ):
        for k in range(NKT):
            blocks = [
                mask[(2 * j + h) * ST:(2 * j + h + 1) * ST, k * ST:(k + 1) * ST]
                for h in (0, 1)
            ]
            if all((b != 0).all() for b in blocks):
                cls[j, k] = 1
            elif all((b == 0).all() for b in blocks):
                cls[j, k] = 0
            else:
                cls[j, k] = 2
        # close interior holes so every slot's computed k-range is contiguous
        nz = np.nonzero(cls[j])[0]
        if len(nz):
            for k in range(nz[0], nz[-1] + 1):
                if cls[j, k] == 0:
                    cls[j, k] = 2
    mixed = [(j, k) for j in range(NJ) for k in range(NKT) if cls[j, k] == 2]
    # dedup mixed tiles by their (h=0, h=1) block content pair: the causal
    # mask yields only 2 distinct patterns, saving SBUF and DMA
    dedup = {}
    midx = {}
    for (j, k) in mixed:
        key = tuple(
            mask[(2 * j + h) * ST:(2 * j + h + 1) * ST,
                 k * ST:(k + 1) * ST].tobytes() for h in (0, 1))
        if key not in dedup:
            dedup[key] = len(dedup)
        midx[(j, k)] = dedup[key]
    return cls, mixed, midx, max(len(dedup), 1)


def _build(cls, mixed, mixed_idx, n_maskt):
    """Build the (uniform, SPMD) Bass program for one core's shard."""
    nc = bacc.Bacc("TRN2", target_bir_lowering=False, debug=False,
                   num_devices=N_CORES)

    x_d = nc.dram_tensor("x", [S, D], BF16, kind="ExternalInput")
    xq_d = nc.dram_tensor("xq", [NJ * ST, D], BF16, kind="ExternalInput")
    wq_d = nc.dram_tensor("wq", [H, D, HD], F32, kind="ExternalInput")
    wk_d = nc.dram_tensor("wk", [H, D, HD], F32, kind="ExternalInput")
    wv_d = nc.dram_tensor("wv", [H, D, HD], F32, kind="ExternalInput")
    wo_d = nc.dram_tensor("wo", [D, D], F32, kind="ExternalInput")
    bq_d = nc.dram_tensor("bq", [H, HD], F32, kind="ExternalInput")
    bk_d = nc.dram_tensor("bk", [H, HD], F32, kind="ExternalInput")
    bv_d = nc.dram_tensor("bv", [H, HD], F32, kind="ExternalInput")
    bo_d = nc.dram_tensor("bo", [D], F32, kind="ExternalInput")
    mt_d = nc.dram_tensor("maskt", [n_maskt, ST, ST], BF16, kind="ExternalInput")
    out_d = nc.dram_tensor("out", [NJ * ST, D], F32, kind="ExternalOutput")

    slots_k = [[j for j in range(NJ) if cls[j, k]] for k in range(NKT)]
    kfirst = {}
    klast = {}
    for j in range(NJ):
        ks = [k for k in range(NKT) if cls[j, k]]
        if ks:
            kfirst[j], klast[j] = ks[0], ks[-1]

    bank_slots = [[j for j in range(4 * b_, 4 * b_ + 4) if j in kfirst]
                  for b_ in range(NB)]
    bklast = {b_: max(klast[j] for j in bank_slots[b_])
              for b_ in range(NB) if bank_slots[b_]}
    bank_fast = {b_: len({kfirst[j] for j in bank_slots[b_]}) == 1
                 for b_ in range(NB) if bank_slots[b_]}

    from concourse.masks import make_identity

    with tile.TileContext(nc) as tc:
        with (
            tc.tile_pool(name="persist", bufs=1) as pp,
        ):
            # ---- persistent SBUF tensors -------------------------------
            kt_t = [pp.tile([ST, S], BF16, name=f"ktg{g}", tag=f"ktg{g}")
                    for g in range(NG)]
            qt_t = [pp.tile([ST, NJ * ST], BF16, name=f"qtg{g}", tag=f"qtg{g}")
                    for g in range(NG)]
            vb = pp.tile([ST, NKT, H, HD + 1], BF16, name="vb", tag="vb")
            cat = [pp.tile([ST, NJ * ST], BF16, name=f"catg{g}", tag=f"catg{g}")
                   for g in range(NG)]
            ident = pp.tile([ST, ST], BF16, name="ident", tag="ident")
            ones1 = pp.tile([1, HD], BF16, name="ones1", tag="ones1")
            mtb = pp.tile([ST, max(n_maskt, 1), ST], BF16, name="mtb",
                          tag="mtb")

            nc.vector.memset(vb[:, :, :, HD:HD + 1], 1.0)
            nc.vector.memset(ones1[:, :], 1.0)
            junk = pp.tile([1, 1], F32, name="junk", tag="junk")
            nc.vector.memset(junk[:, :], 0.0)
            make_identity(nc, ident[:, :])
            nc.scalar.dma_start(mtb[:, :, :],
                                mt_d.ap().rearrange("m p f -> p m f"))

            def load_bias_pair(pool, bias_d, name):
                # [128, NG] f32: partition = (h%2)*64+e, column = pair idx
                t = pool.tile([ST, NG], F32, name=name, tag=name, bufs=1)
                src = bias_d.ap()
                nc.scalar.dma_start(
                    t[:, :],
                    bass.AP(tensor=src.tensor, offset=src.offset,
                            ap=[[1, ST], [ST, NG]]))
                return t

            # x^T tiles (live until the last K/Q projection)
            xtp_cm = tc.tile_pool(name="xtp", bufs=1, side="right")
            xtp = xtp_cm.__enter__()
            xt_all = xtp.tile([ST, NDC, NKT, ST], BF16, name="xt_all",
                              tag="xt_all")
            xqt_all = xtp.tile([ST, NDC, NJ, ST], BF16, name="xqt_all",
                               tag="xqt_all")

            # weight-pair destination pool (lives phase A .. attention)
            wp_cm = tc.tile_pool(name="wpair", bufs=2)
            wp = wp_cm.__enter__()

            bkp = load_bias_pair(pp, bk_d, "bkp")
            bqp = load_bias_pair(pp, bq_d, "bqp")

            def k_proj_block(wpr, g, sg, pool, pbufs=2):
                psk = pool.tile([ST, 512], F32, tag="psk", name="psk",
                                bufs=pbufs)
                for c in range(NDC):
                    nc.tensor.matmul(
                        psk[:, :], wpr[:, c, :],
                        xt_all[:, c, 4 * sg:4 * (sg + 1), :],
                        start=(c == 0), stop=(c == NDC - 1))
                nc.vector.tensor_scalar(
                    kt_t[g][:, sg * 512:(sg + 1) * 512],
                    psk[:, :], bkp[:, g:g + 1], None,
                    mybir.AluOpType.add)

            def q_proj_block(wpr, g, sg, pool, pbufs=2):
                psk = pool.tile([ST, 512], F32, tag="psk", name="psk",
                                bufs=pbufs)
                for c in range(NDC):
                    nc.tensor.matmul(
                        psk[:, :], wpr[:, c, :],
                        xqt_all[:, c, 4 * sg:4 * (sg + 1), :],
                        start=(c == 0), stop=(c == NDC - 1))
                nc.vector.tensor_scalar(
                    qt_t[g][:, sg * 512:(sg + 1) * 512],
                    psk[:, :], bqp[:, g:g + 1], None,
                    mybir.AluOpType.add)

            def stage_pair_weights(w_d, g, tag):
                # casting gpsimd DMAs straight into the [128, NDC, 128]
                # stationary-pair layout (f32 -> bf16 in the DMA)
                wpr = wp.tile([ST, NDC, ST], BF16, name=f"{tag}{g}", tag=tag)
                for h2 in range(2):
                    src = w_d.ap()[2 * g + h2].rearrange(
                        "(c p) e -> p c e", p=ST)
                    nc.gpsimd.dma_start(
                        wpr[:, :, h2 * HD:(h2 + 1) * HD], src)
                return wpr

            # ---- phase A: x^T, xq^T, then a dense V pass ---------------
            with (
                tc.tile_pool(name="p1a", bufs=2) as p1a,
                tc.tile_pool(name="pv", bufs=1) as pv,
                tc.tile_pool(name="ppst", bufs=3, space="PSUM") as ppst,
                tc.tile_pool(name="ppsv", bufs=2, space="PSUM") as ppsv,
            ):
                wvb = pv.tile([ST, NDC, H, HD], BF16, name="wvb", tag="wvb",
                              bufs=1)
                bvf = pv.tile([ST, H, HD], F32, name="bvf", tag="bvf", bufs=1)
                # gpsimd casting-DMA order matches consumption order:
                # V heads 0-7 (n=0 pass), pair-0 K/Q weights, V heads 8-15
                for hh in range(H // 2):
                    srcw = wv_d.ap()[hh].rearrange("(c p) e -> p c e", p=ST)
                    nc.gpsimd.dma_start(wvb[:, :, hh, :], srcw)
                _stage0 = (stage_pair_weights(wk_d, 0, "wkpr"),
                           stage_pair_weights(wq_d, 0, "wqpr"))
                for hh in range(H // 2, H):
                    srcw = wv_d.ap()[hh].rearrange("(c p) e -> p c e", p=ST)
                    nc.gpsimd.dma_start(wvb[:, :, hh, :], srcw)

                evac_i = 0

                def transpose_tile(dst_all, sti, xsrc_ap):
                    nonlocal evac_i
                    xb = p1a.tile([ST, D], BF16, tag="xb", name="xb",
                                  bufs=6)
                    qeng = nc.sync if (evac_i // 8) % 2 == 0 else nc.scalar
                    qeng.dma_start(xb[:, :], xsrc_ap)
                    for c4 in range(NDC // 4):
                        pst4 = ppst.tile([ST, 4, ST], BF16, tag="pst",
                                         name="pst")
                        for i in range(4):
                            c = 4 * c4 + i
                            nc.tensor.transpose(
                                pst4[:, i, :], xb[:, c * ST:(c + 1) * ST],
                                ident[:, :])
                        if evac_i % 2 == 1:
                            nc.scalar.copy(
                                dst_all[:, 4 * c4:4 * c4 + 4, sti, :],
                                pst4[:, :, :])
                        else:
                            nc.vector.tensor_copy(
                                dst_all[:, 4 * c4:4 * c4 + 4, sti, :],
                                pst4[:, :, :])
                        evac_i += 1

                for st in range(NKT):
                    transpose_tile(xt_all, st,
                                   x_d.ap()[st * ST:(st + 1) * ST, :])
                    if st == 0:
                        srcv = bv_d.ap()
                        nc.scalar.dma_start(
                            bvf[:, :, :],
                            bass.AP(tensor=srcv.tensor, offset=srcv.offset,
                                    ap=[[0, ST]] + list(srcv.ap)))

                for jl in range(NJ):
                    transpose_tile(xqt_all, jl,
                                   xq_d.ap()[jl * ST:(jl + 1) * ST, :])

                # preload the Exp activation table while ScalarE is idle
                nc.scalar.activation(junk[:, :], junk[:, :],
                                     mybir.ActivationFunctionType.Exp)
                _p0 = ([(k_proj_block, _stage0[0], sg, ppst)
                        for sg in range(NSG)] +
                       [(q_proj_block, _stage0[1], sg, ppst)
                        for sg in range(NQG)])

                # dense V projection in two head-half passes so the n=0
                # pass only needs the first 8 heads' weights (arriving
                # early on the gpsimd queue); pair-0 K/Q blocks interleave
                for n in range(2):
                    for st in range(NKT):
                        psv = ppsv.tile([ST, 8 * HD], F32, tag="psv",
                                        name="psv")
                        for c in range(NDC):
                            nc.tensor.matmul(
                                psv[:, :],
                                xt_all[:, c, st, :],
                                wvb[:, c, 8 * n:8 * n + 8, :],
                                start=(c == 0), stop=(c == NDC - 1))
                        nc.vector.tensor_add(
                            vb[:, st, 8 * n:8 * n + 8, 0:HD],
                            psv[:, :].rearrange("p (h e) -> p h e", h=8),
                            bvf[:, 8 * n:8 * n + 8, :])
                        if st % 3 == 2 and _p0:
                            fn, w_, sg_, pl_ = _p0.pop(0)
                            fn(w_, 0, sg_, pl_, 1)
                while _p0:
                    fn, w_, sg_, pl_ = _p0.pop(0)
                    fn(w_, 0, sg_, pl_, 1)

            # ---- phases B/C/D: projections + attention + fc_out --------
            # PSUM budget: psc 2x2 banks + po 2x1 banks + psk 2x1 = 8.
            p2s_cm = tc.tile_pool(name="p2s", bufs=2)
            p2s = p2s_cm.__enter__()
            wop_cm = tc.tile_pool(name="wop", bufs=1)
            wop = wop_cm.__enter__()
            wob = wop.tile([ST, NDC, D], BF16, name="wob", tag="wob")
            bob = wop.tile([ST, D], BF16, name="bob", tag="bob")
            bo_ap = bo_d.ap()
            nc.gpsimd.dma_start(
                bob[:, :],
                bass.AP(tensor=bo_ap.tensor, offset=bo_ap.offset,
                        ap=[[0, ST]] + list(bo_ap.ap)))
            pt_cm = tc.tile_pool(name="ptp", bufs=6)
            ptp = pt_cm.__enter__()
            psc_cm = tc.tile_pool(name="psc", bufs=2, space="PSUM")
            pscp = psc_cm.__enter__()
            po_cm = tc.tile_pool(name="po", bufs=2, space="PSUM")
            pop = po_cm.__enter__()
            psk_cm = tc.tile_pool(name="psk", bufs=2, space="PSUM")
            pskp = psk_cm.__enter__()

            def proj_chunks_for_pair(g, staged=None):
                # closures emitting one tensor-engine chunk each
                if staged is None:
                    wk_pr = stage_pair_weights(wk_d, g, "wkpr")
                    wq_pr = stage_pair_weights(wq_d, g, "wqpr")
                else:
                    wk_pr, wq_pr = staged
                chunks = []
                for sg in range(NSG):
                    chunks.append(
                        lambda sg=sg: k_proj_block(wk_pr, g, sg, pskp))
                for sg in range(NQG):
                    chunks.append(
                        lambda sg=sg: q_proj_block(wq_pr, g, sg, pskp))
                return chunks

            _pair0_chunks = []

            def wo_chunks(cs):
                chunks = []
                for c in cs:
                    def ch(c=c):
                        nc.gpsimd.dma_start(wob[:, c, :],
                                            wo_d.ap()[c * ST:(c + 1) * ST, :])
                    chunks.append(ch)
                return chunks

            # pair 0 was projected during the phase-A V pass

            def attention_head(g, h, pending_chunks, prev_fin):
                """Emit attention for head h (pair g).

                pending_chunks: proj/wo closures drained ~evenly into the
                k-loop.  prev_fin: deferred normalization-finalize closures
                of the previous head, drained after the first k-iterations.
                Returns this head's own finalize closures."""
                r = (h % 2) * HD
                po = {}
                for b_ in range(NB):
                    if bank_slots[b_]:
                        po[b_] = pop.tile([HD + 1, 512], F32, tag="po",
                                          name=f"po{h}_{b_}")
                        if not bank_fast[b_]:
                            nc.vector.memset(po[b_][:, :], 0.0)

                active_ks = [k for k in range(NKT) if slots_k[k]]
                n_it = max(1, (len(active_ks) * 3) // 4)
                drain_every = max(1, n_it // (len(pending_chunks) + 1)) \
                    if pending_chunks else 0

                norm_state = {}

                def norm_pre(b_):
                    # PSUM-side reads: frees the po slot early; no tensor op
                    ltmp = p2s.tile([1, 512], F32, tag="ltmp", name="ltmp",
                                    bufs=1)
                    nc.vector.tensor_copy(ltmp[:, :], po[b_][HD:HD + 1, :])
                    rec = p2s.tile([1, 512], F32, tag="rec", name="rec",
                                   bufs=1)
                    nc.vector.reciprocal_approx_fast(rec[:, :], ltmp[:, :])
                    rec16 = p2s.tile([1, 512], BF16, tag="rec16", name="rec16",
                                     bufs=2)
                    nc.vector.tensor_copy(rec16[:, :], rec[:, :])
                    cslice = cat[g][r:r + HD, 4 * b_ * ST:(4 * b_ + 4) * ST]
                    nc.vector.tensor_copy(cslice, po[b_][0:HD, :])
                    norm_state[b_] = (rec16, cslice)

                def norm_fin(b_):
                    rec16, cslice = norm_state[b_]
                    recps = pskp.tile([HD, 512], F32, tag="psk", name="recps")
                    nc.tensor.matmul(recps[:, :], ones1[:, :], rec16[:, :],
                                     start=True, stop=True)
                    nc.vector.tensor_mul(cslice, cslice, recps[:, :])

                def emit_av(item, paired, pt):
                    done_banks = []
                    for par, (k, runs) in enumerate(item):
                        for run in runs:
                            sub = [run[0]]
                            subs = []
                            for j in run[1:]:
                                if kfirst[j] == kfirst[sub[0]]:
                                    sub.append(j)
                                else:
                                    subs.append(sub)
                                    sub = [j]
                            subs.append(sub)
                            for sub_ in subs:
                                ja, jb = sub_[0], sub_[-1]
                                b_ = ja // 4
                                fast = bank_fast[b_]
                                co = (par * 512 + (ja - 4 * b_) * ST
                                      if paired else ja * ST)
                                nc.tensor.matmul(
                                    po[b_][0:HD + 1,
                                           (ja - 4 * b_) * ST:
                                           (jb + 1 - 4 * b_) * ST],
                                    vb[:, k, h, :],
                                    pt[:, co:co + (jb + 1 - ja) * ST],
                                    start=(fast and k == kfirst[ja]),
                                    stop=(fast and k == bklast[b_]),
                                    skip_group_check=not fast)
                                if fast and k == bklast[b_]:
                                    done_banks.append(b_)
                    return done_banks

                def runs_of(sl):
                    runs = []
                    run = [sl[0]]
                    for j in sl[1:]:
                        if j == run[-1] + 1 and j // 4 == run[0] // 4:
                            run.append(j)
                        else:
                            runs.append(run)
                            run = [j]
                    runs.append(run)
                    return runs

                def is_narrow(runs):
                    return (len(runs) == 1 and
                            (runs[0][-1] - runs[0][0] + 1) * ST <= 512)

                # batch: pair up narrow k-tiles (single run <= 512 wide) so
                # one exp ACTIVATE serves two k-tiles
                items = []
                i = 0
                while i < len(active_ks):
                    k = active_ks[i]
                    rk = runs_of(slots_k[k])
                    if is_narrow(rk) and i + 1 < len(active_ks):
                        k2 = active_ks[i + 1]
                        rk2 = runs_of(slots_k[k2])
                        if is_narrow(rk2):
                            items.append([(k, rk), (k2, rk2)])
                            i += 2
                            continue
                    items.append([(k, rk)])
                    i += 1

                def colof(j, par, paired):
                    # flat column of slot j within psc/pt for this sub-tile
                    if paired:
                        return par * 512 + (j - 4 * (j // 4)) * ST
                    return j * ST

                pending = []
                for ii, item in enumerate(items):
                    paired = len(item) == 2
                    psc = pscp.tile([ST, NJ * ST], F32, tag="psc", name="psc")
                    for par, (k, runs) in enumerate(item):
                        for run in runs:
                            ja, jb = run[0], run[-1]
                            w = (jb + 1 - ja) * ST
                            co = colof(ja, par, paired)
                            nc.tensor.matmul(
                                psc[:, co:co + w],
                                kt_t[g][r:r + HD, k * ST:(k + 1) * ST],
                                qt_t[g][r:r + HD, ja * ST:(jb + 1) * ST],
                                start=True, stop=True)
                    pt = ptp.tile([ST, NJ * ST], BF16, tag="pt", name="pt")
                    if paired:
                        # one exp over both sub-tiles via a strided 3-dim AP
                        o0 = min(colof(k_r[0][0], 0, True)
                                 for (kk, k_r) in item) % 512
                        o1 = max(colof(k_r[0][-1], 0, True) % 512 + ST
                                 for (kk, k_r) in item)
                        psc2 = psc[:, :].rearrange("p (a c) -> p a c", a=2)
                        pt2 = pt[:, :].rearrange("p (a c) -> p a c", a=2)
                        nc.scalar.activation(
                            pt2[:, :, o0:o1], psc2[:, :, o0:o1],
                            mybir.ActivationFunctionType.Exp,
                            scale=1.0 / float(np.sqrt(HD)))
                    else:
                        k, runs = item[0]
                        sl = slots_k[k]
                        jaT, jbT = sl[0], sl[-1]
                        nc.scalar.activation(
                            pt[:, jaT * ST:(jbT + 1) * ST],
                            psc[:, jaT * ST:(jbT + 1) * ST],
                            mybir.ActivationFunctionType.Exp,
                            scale=1.0 / float(np.sqrt(HD)))
                    for par, (k, runs) in enumerate(item):
                        for j in slots_k[k]:
                            if cls[j, k] == 2:
                                m = mixed_idx[(j, k)]
                                co = colof(j, par, paired)
                                nc.vector.tensor_mul(
                                    pt[:, co:co + ST],
                                    pt[:, co:co + ST],
                                    mtb[:, m, :])
                    pending.append((item, paired, pt))
                    if len(pending) > 1:
                        for b_ in emit_av(*pending.pop(0)):
                            norm_pre(b_)
                            if b_ != NB - 1:
                                # groups completing mid-head finalize in-head
                                norm_fin(b_)
                    if prev_fin and ii == 2:
                        while prev_fin:
                            prev_fin.pop(0)()
                    if pending_chunks and drain_every and \
                            ii % drain_every == drain_every - 1:
                        pending_chunks.pop(0)()
                for args in pending:
                    for b_ in emit_av(*args):
                        norm_pre(b_)
                        if b_ != NB - 1:
                            norm_fin(b_)
                while pending_chunks:
                    pending_chunks.pop(0)()
                # slow path for masks where a group never hits bklast (not
                # bank_fast): normalize any group not yet handled
                fins = []
                for b_ in range(NB):
                    if bank_slots[b_] and b_ not in norm_state:
                        norm_pre(b_)
                        if b_ != NB - 1:
                            norm_fin(b_)
                # the last group's tensor finalize is deferred into the
                # next head's stream (returned to the caller)
                if bank_slots[NB - 1]:
                    fins.append(lambda: norm_fin(NB - 1))
                return fins

            fins = []
            for g in range(NG):
                chunks = proj_chunks_for_pair(g + 1) if g + 1 < NG else []
                if g == NG - 3:
                    chunks += wo_chunks(range(0, 4))
                if g == NG - 2:
                    chunks += wo_chunks(range(4, NDC))
                # split interleaved chunks between the two heads
                half = (len(chunks) + 1) // 2
                fins = attention_head(g, 2 * g, chunks[:half], fins)
                fins += attention_head(g, 2 * g + 1, chunks[half:], fins)
            for f in fins:
                f()

            psk_cm.__exit__(None, None, None)
            po_cm.__exit__(None, None, None)
            psc_cm.__exit__(None, None, None)
            pt_cm.__exit__(None, None, None)

            # ---- phase D: fc_out ---------------------------------------
            with (
                tc.tile_pool(name="p3s", bufs=3) as p3s,
                tc.tile_pool(name="psy", bufs=4, space="PSUM") as psy,
            ):
                for jt in range(NJ):
                    py = [psy.tile([ST, 512], F32, tag="py",
                                   name=f"py{jt}_{n}") for n in range(2)]
                    for c in range(NDC):
                        for n in range(2):
                            nc.tensor.matmul(
                                py[n][:, :],
                                cat[c][:, jt * ST:(jt + 1) * ST],
                                wob[:, c, n * 512:(n + 1) * 512],
                                start=(c == 0), stop=(c == NDC - 1))
                    for n in range(2):
                        ysb = p3s.tile([ST, 512], F32, tag="ysb", name="ysb")
                        nc.vector.tensor_add(ysb[:, :], py[n][:, :],
                                             bob[:, n * 512:(n + 1) * 512])
                        nc.sync.dma_start(
                            out_d.ap()[jt * ST:(jt + 1) * ST,
                                       n * 512:(n + 1) * 512],
                            ysb[:, :])

            wop_cm.__exit__(None, None, None)
            p2s_cm.__exit__(None, None, None)
            wp_cm.__exit__(None, None, None)
            xtp_cm.__exit__(None, None, None)

    nc.compile()
    return nc


_CACHE = {}
LAST_RESULT = None


def _get_program(mask):
    key = mask.tobytes()
    if key not in _CACHE:
        cls, mixed, midx, n_maskt = _classify(mask)
        _CACHE[key] = (_build(cls, mixed, midx, n_maskt), cls, mixed, midx,
                       n_maskt)
    return _CACHE[key]


def kernel(x, mask, Wq, bq, Wk, bk, Wv, bv, Wo, bo):
    x = np.ascontiguousarray(np.asarray(x, dtype=np.float32))
    mask = np.asarray(mask)
    nc, cls, mixed, midx, n_maskt = _get_program(mask)

    base = {
        "wq": np.ascontiguousarray(Wq, dtype=np.float32),
        "wk": np.ascontiguousarray(Wk, dtype=np.float32),
        "wv": np.ascontiguousarray(Wv, dtype=np.float32),
        "wo": np.ascontiguousarray(Wo, dtype=np.float32),
        "bq": np.ascontiguousarray(bq, dtype=np.float32),
        "bk": np.ascontiguousarray(bk, dtype=np.float32),
        "bv": np.ascontiguousarray(bv, dtype=np.float32),
        "bo": np.ascontiguousarray(bo, dtype=np.float32),
    }
    in_maps = []
    for c in range(N_CORES):
        b, h = c // 2, c % 2
        qrows = np.concatenate(
            [np.arange((2 * j + h) * ST, (2 * j + h + 1) * ST) for j in range(NJ)])
        mt = np.zeros((n_maskt, ST, ST), dtype=ml_dtypes.bfloat16)
        for (j, k) in mixed:
            blk = mask[(2 * j + h) * ST:(2 * j + h + 1) * ST,
                       k * ST:(k + 1) * ST]
            mt[midx[(j, k)]] = (blk != 0).T.astype(ml_dtypes.bfloat16)
        m = dict(base)
        m["x"] = np.ascontiguousarray(x[b].astype(ml_dtypes.bfloat16))
        m["xq"] = np.ascontiguousarray(
            x[b][qrows].astype(ml_dtypes.bfloat16))
        m["maskt"] = mt
        in_maps.append(m)

    res = run_bass_kernel_spmd(
        nc, in_maps, core_ids=list(range(N_CORES)),
        trace=os.environ.get("BASS_KERNEL_TRACE", "0") == "1")
    global LAST_RESULT
    LAST_RESULT = res

    out = np.empty((B, S, D), dtype=np.float32)
    for c in range(N_CORES):
        b, h = c // 2, c % 2
        oc = res.results[c]["out"]
        for j in range(NJ):
            out[b, (2 * j + h) * ST:(2 * j + h + 1) * ST, :] = \
                oc[j * ST:(j + 1) * ST, :]
    return out
